# revision 1
# baseline (speedup 1.0000x reference)
"""Detection-criterion loss kernel for Trainium2 (8 NeuronCores, SPMD).

loss = 2*class_bce + 4*xywh_sse + obj_bce   summed over 6M (batch*anchor) rows.

Math trick: for a binary target t and prob p,
    t*log(p) + (1-t)*log(1-p) = log|p + t - 1| = 0.5 * log((t_bar - p)^2)
with t_bar = 1 - t. So each BCE term needs one subtract, one square, one log
-- no select. The one-hot class target is fused into the subtract via
scalar_tensor_tensor: s_j = (cls != j) - p_j  (j = 0..2), and the weighted sum
2*class + obj is recovered at the end from separately reduced regions:
    total = 4*sum(d^2) - sum(ln(prod_j s_j^2)) - 0.5*sum(ln(s_obj^2)).

Per-core layout (25 tiles of 128 partitions x 235 rows):
    X = [ d (3R) | s_cls planar (3R) | s_obj (R) ]   (DVE, 5 ops)
    Y[0:7R] = Square(X)                              (ACT, 1 op)
    Y[7R:8R] = P2 = qc0*qc1*qc2                      (GPSIMD, 2 ops)
    LL = Ln(Y[6R:8R]) = [ln q_obj^2 | ln P2]         (ACT, 1 op)
    psum += ones.T @ [Y[0:3R], LL]                   (PE matmuls, accumulating)
Host sums the 8 per-core [1, 5R] partial vectors in float64.
"""

import numpy as np

P = 128                  # SBUF partitions
R = 235                  # rows per partition per tile
TILE_ROWS = P * R        # 30080
T = 25                   # tiles per core
CORE_ROWS = T * TILE_ROWS  # 752000
N_CORES = 8
TOTAL_ROWS = 2_000_000 * 3

_CACHE = {}


def _build_module(reps: int = 1, io_bufs: int = 3, work_bufs: int = 2):
    import concourse.bacc as bacc
    import concourse.bass as bass
    import concourse.tile as tile
    from concourse import mybir

    f32 = mybir.dt.float32
    AF = mybir.ActivationFunctionType
    OP = mybir.AluOpType

    nc = bacc.Bacc(None, target_bir_lowering=False)

    o_d = nc.dram_tensor("o", [CORE_ROWS, 7], f32, kind="ExternalInput")
    g_d = nc.dram_tensor("g", [CORE_ROWS, 5], f32, kind="ExternalInput")
    res_d = nc.dram_tensor("res", [1, 5 * R], f32, kind="ExternalOutput")

    R3 = 3 * R

    with tile.TileContext(nc) as tc:
        with (
            tc.tile_pool(name="io", bufs=io_bufs) as io,
            tc.tile_pool(name="work", bufs=work_bufs) as work,
            tc.tile_pool(name="consts", bufs=1) as consts,
            tc.tile_pool(name="ps", bufs=1, space=bass.MemorySpace.PSUM) as ps,
        ):
            ones = consts.tile([P, 1], f32)
            nc.vector.memset(ones[:], 1.0)

            psum_sq = ps.tile([1, R3], f32)      # sum of d^2 per free slot
            psum_ll = ps.tile([1, 2 * R], f32)   # [ln q_obj^2 | ln P2] sums

            for rep, t in ((rp, tt) for rp in range(reps) for tt in range(T)):
                lo = t * TILE_ROWS
                so = io.tile([P, R, 7], f32, tag="so")
                sg = io.tile([P, R, 5], f32, tag="sg")
                nc.sync.dma_start(
                    out=so[:],
                    in_=o_d[lo : lo + TILE_ROWS, :].rearrange(
                        "(p j) c -> p j c", p=P
                    ),
                )
                nc.sync.dma_start(
                    out=sg[:],
                    in_=g_d[lo : lo + TILE_ROWS, :].rearrange(
                        "(p j) c -> p j c", p=P
                    ),
                )

                x = work.tile([P, 7 * R], f32, tag="x")
                y = work.tile([P, 8 * R], f32, tag="y")
                ll = work.tile([P, 2 * R], f32, tag="ll")
                p1 = work.tile([P, R], f32, tag="p1")

                # coord diffs, planar [c-major] so Y[0:3R] is dense
                nc.vector.tensor_sub(
                    x[:, 0:R3].rearrange("p (c r) -> p r c", c=3),
                    so[:, :, 1:4],
                    sg[:, :, 1:4],
                )
                # class terms: s_j = (cls != j) - p_j
                for j in range(3):
                    nc.vector.scalar_tensor_tensor(
                        out=x[:, R3 + j * R : R3 + (j + 1) * R],
                        in0=sg[:, :, 4],
                        scalar=float(j),
                        in1=so[:, :, 4 + j],
                        op0=OP.not_equal,
                        op1=OP.subtract,
                    )
                # obj term: s = (obj == 0) - p_obj  (= (1-t) - p)
                nc.vector.scalar_tensor_tensor(
                    out=x[:, 6 * R : 7 * R],
                    in0=sg[:, :, 0],
                    scalar=0.0,
                    in1=so[:, :, 0],
                    op0=OP.is_equal,
                    op1=OP.subtract,
                )
                # squares: Y[0:3R]=d^2, [3R:6R]=qc2, [6R:7R]=q_obj^2
                nc.scalar.activation(y[:, 0 : 7 * R], x[:, 0 : 7 * R], AF.Square)
                # class product P2 = qc2_0 * qc2_1 * qc2_2 -> Y[7R:8R]
                nc.gpsimd.tensor_mul(p1[:], y[:, R3 : 4 * R], y[:, 4 * R : 5 * R])
                nc.gpsimd.tensor_mul(y[:, 7 * R : 8 * R], p1[:], y[:, 5 * R : 6 * R])
                # logs over contiguous [q_obj^2 | P2]
                nc.scalar.activation(ll[:], y[:, 6 * R : 8 * R], AF.Ln)

                # reductions over partitions, accumulated over tiles in PSUM
                st = (t == 0) and (rep == 0)
                sp = (t == T - 1) and (rep == reps - 1)
                nc.tensor.matmul(
                    psum_sq[:, 0:512], ones[:], y[:, 0:512], start=st, stop=sp
                )
                nc.tensor.matmul(
                    psum_sq[:, 512:R3], ones[:], y[:, 512:R3], start=st, stop=sp
                )
                nc.tensor.matmul(
                    psum_ll[:], ones[:], ll[:], start=st, stop=sp
                )

            out_sb = consts.tile([1, 5 * R], f32)
            nc.vector.tensor_copy(out_sb[:, 0:R3], psum_sq[:])
            nc.vector.tensor_copy(out_sb[:, R3 : 5 * R], psum_ll[:])
            nc.sync.dma_start(res_d[:], out_sb[:])

    nc.compile()
    return nc


def _get_module(reps: int = 1, io_bufs: int = 3, work_bufs: int = 2):
    key = ("nc", reps, io_bufs, work_bufs)
    if key not in _CACHE:
        _CACHE[key] = _build_module(reps, io_bufs, work_bufs)
    return _CACHE[key]


def kernel(output: np.ndarray, target: np.ndarray) -> np.ndarray:
    from concourse.bass_utils import run_bass_kernel_spmd

    o = np.ascontiguousarray(output, dtype=np.float32).reshape(TOTAL_ROWS, 7)
    g = np.ascontiguousarray(target, dtype=np.float32).reshape(TOTAL_ROWS, 5)

    in_maps = []
    for c in range(N_CORES):
        lo = c * CORE_ROWS
        hi = min(lo + CORE_ROWS, TOTAL_ROWS)
        oc, gc = o[lo:hi], g[lo:hi]
        if hi - lo < CORE_ROWS:
            padn = CORE_ROWS - (hi - lo)
            opad = np.zeros((padn, 7), np.float32)
            gpad = np.zeros((padn, 5), np.float32)
            gpad[:, 4] = -1.0  # class id outside [0,3) -> zero loss contribution
            oc = np.concatenate([oc, opad])
            gc = np.concatenate([gc, gpad])
        in_maps.append({"o": oc, "g": gc})

    nc = _get_module()
    r = run_bass_kernel_spmd(nc, in_maps, core_ids=list(range(N_CORES)))

    R3 = 3 * R
    total = 0.0
    for c in range(N_CORES):
        res = np.asarray(r.results[c]["res"]).reshape(-1).astype(np.float64)
        total += (
            4.0 * res[0:R3].sum()
            - 0.5 * res[R3 : 4 * R].sum()
            - res[4 * R : 5 * R].sum()
        )
    return np.array(total, dtype=np.float32)



# revision 2
# speedup vs baseline: 1.0931x; 1.0931x over previous
"""Detection-criterion loss kernel for Trainium2 (8 NeuronCores, SPMD).

loss = 2*class_bce + 4*xywh_sse + obj_bce   summed over 6M (batch*anchor) rows.

Math trick: for a binary target t and prob p,
    t*log(p) + (1-t)*log(1-p) = log|p + t - 1| = 0.5 * log((t_bar - p)^2)
with t_bar = 1 - t. So each BCE term needs one subtract, one square, one log
-- no select. The one-hot class target is fused into the subtract via
scalar_tensor_tensor: s_j = (cls != j) - p_j  (j = 0..2), and the weighted sum
is recovered from three separately accumulated sums:
    total = 4*sum(d^2) - sum(ln q_cls^2) - 0.5*sum(ln q_obj^2).

Per-core layout (25 tiles of 128 partitions x 235 rows):
    X[0:3R]  = d   (coords diff, interleaved)   (GPSIMD tensor_sub, 1 op)
    X[3R:6R] = s_cls planar                     (DVE stt, 3 ops)
    X[6R:7R] = s_obj                            (DVE stt, 1 op)
    Y        = Square(X[3R:7R]) -> bf16         (ACT)
    Square(X[0:3R])  accum-> acc[:, t]          (ACT, accum_out)
    Ln(Y[0:3R])      accum-> acc[:, T+t]        (ACT, accum_out)
    Ln(Y[3R:4R])     accum-> acc[:, 2T+t]       (ACT, accum_out)
No PE matmuls: reductions ride the activation accum_out path; the final
[128, 3T] partials are summed on the host in float64.
"""

import numpy as np

P = 128                  # SBUF partitions
R = 235                  # rows per partition per tile
TILE_ROWS = P * R        # 30080
T = 25                   # tiles per core
CORE_ROWS = T * TILE_ROWS  # 752000
N_CORES = 8
TOTAL_ROWS = 2_000_000 * 3

_CACHE = {}


def _build_module(io_bufs: int = 4, work_bufs: int = 2):
    import concourse.bacc as bacc
    import concourse.bass as bass
    import concourse.tile as tile
    from concourse import mybir

    f32 = mybir.dt.float32
    bf16 = mybir.dt.bfloat16
    AF = mybir.ActivationFunctionType
    OP = mybir.AluOpType

    nc = bacc.Bacc(None, target_bir_lowering=False)

    o_d = nc.dram_tensor("o", [CORE_ROWS, 7], f32, kind="ExternalInput")
    g_d = nc.dram_tensor("g", [CORE_ROWS, 5], f32, kind="ExternalInput")
    res_d = nc.dram_tensor("res", [P, 3 * T], f32, kind="ExternalOutput")

    R3 = 3 * R
    R4 = 4 * R

    with tile.TileContext(nc) as tc:
        with (
            tc.tile_pool(name="io", bufs=io_bufs) as io,
            tc.tile_pool(name="work", bufs=work_bufs) as work,
            tc.tile_pool(name="consts", bufs=1) as consts,
        ):
            acc = consts.tile([P, 3 * T], f32)

            for t in range(T):
                lo = t * TILE_ROWS
                so = io.tile([P, R, 7], f32, tag="so")
                sg = io.tile([P, R, 5], f32, tag="sg")
                nc.sync.dma_start(
                    out=so[:],
                    in_=o_d[lo : lo + TILE_ROWS, :].rearrange(
                        "(p j) c -> p j c", p=P
                    ),
                )
                nc.sync.dma_start(
                    out=sg[:],
                    in_=g_d[lo : lo + TILE_ROWS, :].rearrange(
                        "(p j) c -> p j c", p=P
                    ),
                )

                x = work.tile([P, 7 * R], f32, tag="x")
                y = work.tile([P, R4], bf16, tag="y")
                scr = work.tile([P, R3], bf16, tag="scr")

                # class terms: s_j = (cls != j) - p_j  (DVE)
                for j in range(3):
                    nc.vector.scalar_tensor_tensor(
                        out=x[:, R3 + j * R : R3 + (j + 1) * R],
                        in0=sg[:, :, 4],
                        scalar=float(j),
                        in1=so[:, :, 4 + j],
                        op0=OP.not_equal,
                        op1=OP.subtract,
                    )
                # obj term: s = (obj == 0) - p_obj  (= (1-t) - p)  (DVE)
                nc.vector.scalar_tensor_tensor(
                    out=x[:, 6 * R : 7 * R],
                    in0=sg[:, :, 0],
                    scalar=0.0,
                    in1=so[:, :, 0],
                    op0=OP.is_equal,
                    op1=OP.subtract,
                )
                # coord diffs, interleaved layout (GPSIMD)
                nc.gpsimd.tensor_sub(
                    x[:, 0:R3].rearrange("p (r c) -> p r c", c=3),
                    so[:, :, 1:4],
                    sg[:, :, 1:4],
                )

                # squares of BCE terms -> bf16
                nc.scalar.activation(y[:], x[:, R3 : 7 * R], AF.Square)
                # sum ln(q_cls^2)
                nc.scalar.activation(
                    scr[:, 0:R3], y[:, 0:R3], AF.Ln,
                    accum_out=acc[:, T + t : T + t + 1],
                )
                # sum ln(q_obj^2)
                nc.scalar.activation(
                    scr[:, 0:R], y[:, R3:R4], AF.Ln,
                    accum_out=acc[:, 2 * T + t : 2 * T + t + 1],
                )
                # sum d^2
                nc.scalar.activation(
                    scr[:, 0:R3], x[:, 0:R3], AF.Square,
                    accum_out=acc[:, t : t + 1],
                )

            nc.sync.dma_start(res_d[:, :], acc[:])

    nc.compile()
    return nc


def _get_module(io_bufs: int = 4, work_bufs: int = 2):
    key = ("nc", io_bufs, work_bufs)
    if key not in _CACHE:
        _CACHE[key] = _build_module(io_bufs, work_bufs)
    return _CACHE[key]


def kernel(output: np.ndarray, target: np.ndarray) -> np.ndarray:
    from concourse.bass_utils import run_bass_kernel_spmd

    o = np.ascontiguousarray(output, dtype=np.float32).reshape(TOTAL_ROWS, 7)
    g = np.ascontiguousarray(target, dtype=np.float32).reshape(TOTAL_ROWS, 5)

    in_maps = []
    for c in range(N_CORES):
        lo = c * CORE_ROWS
        hi = min(lo + CORE_ROWS, TOTAL_ROWS)
        oc, gc = o[lo:hi], g[lo:hi]
        if hi - lo < CORE_ROWS:
            padn = CORE_ROWS - (hi - lo)
            opad = np.zeros((padn, 7), np.float32)
            gpad = np.zeros((padn, 5), np.float32)
            gpad[:, 4] = -1.0  # class id outside [0,3) -> zero loss contribution
            oc = np.concatenate([oc, opad])
            gc = np.concatenate([gc, gpad])
        in_maps.append({"o": oc, "g": gc})

    nc = _get_module()
    r = run_bass_kernel_spmd(nc, in_maps, core_ids=list(range(N_CORES)))

    total = 0.0
    for c in range(N_CORES):
        res = np.asarray(r.results[c]["res"]).astype(np.float64)
        sums = res.sum(axis=0)  # [3T]
        total += (
            4.0 * sums[0:T].sum()
            - sums[T : 2 * T].sum()
            - 0.5 * sums[2 * T : 3 * T].sum()
        )
    return np.array(total, dtype=np.float32)
